# revision 1
# baseline (speedup 1.0000x reference)
"""Boundary-distance loss (BDLoss) on 8 Trainium2 NeuronCores.

Math (matches the reference):
  probs = softmax(net_output, axis=1)
  onehot_c = (gt == c)
  posdis = EDT(onehot_c), negdis = EDT(~onehot_c)
  phi = where(inner_boundary, 0, negdis - posdis), zeroed if class absent
  out  = mean(probs[:, 1:] * phi[:, 1:])

Key algorithmic structure of this implementation:
  * channel 0 never contributes -> only classes 1..3 are computed.
  * NEG field (D=2 windowed separable squared EDT, exact while neg2 <= 8):
    the x-pass runs as ONE matmul per chunk on the foreground mask with the
    complement folded in: s_old = 26 - r with r = W_neg @ fpos, so the decode
    thresholds flip to is_le and no complement tensor is ever built.  The y/z
    passes are pair-mins + cheap 4x-mode +d^2 tensor_scalar biases + mins,
    all on DVE (cross-engine hops lose more to queue serialization than the
    engine offload gains; measured, repeatedly).
  * POS field: for the voxels that matter (pos2 <= 3, host-verified), the
    boundary-zeroed positive distance is a pure function of the counts of
    foreground face/edge/corner neighbors:
       s = 256*c1 + 16*c2 + c3   (c1<=6, c2<=12, c3<=8, max 1736: fp16-exact)
       posd' = sqrt2*[s>=1536] + (sqrt3-sqrt2)*[s>=1728], gated by fpos.
    s is computed with 9 banded matmuls (one per (dy,dz) tap, x-taps in the
    band) + 1 rank-4 correction matmul for out-of-volume reads.  This replaces
    the pos x/y/z passes, the boundary-indicator ops and the pos sqrt.
  * Host-side verification: pos exactness (no foreground voxel whose full
    3^3 neighborhood is foreground) is checked with numpy erosion on gt;
    neg exactness is checked from the shipped nz field (max <= 8.5; the
    windowed pass yields >= 12 wherever it would be inexact).  On failure the
    host falls back to an exact scipy computation.
  * softmax tail is bf16: exp(net) -> e (ACT), den = sum_c e_c and the
    1/den folds run on GpSimd, inv = exp(-ln den) on ACT; each class is
    reduced separately via the ACT accumulator into its own output column.
    Engine layout: DVE decodes/mins (the bound, ~86% busy), ACT exps/copies/
    sqrt/reduce, PE all matmuls, GpSimd masks(2,3)/den/1-over-den folds.

Sharding: core = (b, z-slab): b = core//4, z0 = 24*(core%4).  gt is sent with
a 2-plane z halo and a 2-column y pad, both filled with class 255 (reads as
"not this class" -> background for fpos, foreground for the neg field, with
out-of-volume taps of the pos radix repaired by the correction matmul).
"""

import numpy as np
import ml_dtypes

import concourse.bacc as bacc
import concourse.mybir as mybir
from concourse.tile import TileContext
from concourse import bass_utils

F32 = mybir.dt.float32
BF16 = mybir.dt.bfloat16
FP16 = mybir.dt.float16
U8 = mybir.dt.uint8
AL = mybir.AluOpType
AF = mybir.ActivationFunctionType

B, C, X, Y, Z = 2, 4, 128, 128, 96
ZO = 24            # output z-planes per core
H = 2              # z halo (= D_neg)
ZT = ZO + 2 * H    # 28 z-planes held on chip
YP = Y + 4         # y padded to 132 columns (2 each side)
FDH = YP * ZT      # 3696 free elems of a padded halo tile
FDO = Y * ZO       # 3072 free elems of a dense output tile
BIGN = 8.0         # f1 "no candidate" extra: 1+3+8 = 12 > 8 (valid windowed B)
NVOX = B * (C - 1) * X * Y * Z      # denominator of the global mean
SQ2 = float(np.sqrt(2.0))
SQ3 = float(np.sqrt(3.0))

# neg x-pass chunking (free-dim cols of the padded tile), aligned with the
# gt DMA / mask chunks so the pipeline starts as soon as chunk 0 lands
NEG_CHUNKS = (1232, 1232, 1232)
# pos radix y-blocks: 21 y-cols = 504 psum cols — each block lives entirely
# inside ONE 2KB psum bank (a matmul output region must not cross a bank)
POS_YBLK = 21
# taps of the pos radix: (dy, dz) -> which band matrix (0=A, 1=B, 2=C)
POS_TAPS = [(0, 0, 0),
            (-1, 0, 1), (1, 0, 1), (0, -1, 1), (0, 1, 1),
            (-1, -1, 2), (-1, 1, 2), (1, -1, 2), (1, 1, 2)]
# per-band (fullw, edgew): band total weight / weight of its dx=+-1 entry
BAND_FULLW = (512.0, 288.0, 18.0)
BAND_EDGEW = (256.0, 16.0, 1.0)
EW_SUM = 256.0 + 4 * 16.0 + 4 * 1.0   # 324: sum of edgew over all 9 taps


DEBUG_POS = False


def _body(tc, gt_d, net_d, wn_d, wpos_d, cw_d, crows_d, out_d, nz_d,
          sdbg_d=None, gdbg_d=None):
    nc = tc.nc
    with tc.tile_pool(name="main", bufs=1) as pool, \
         tc.tile_pool(name="rot", bufs=2) as rot, \
         tc.tile_pool(name="ps", bufs=2, space="PSUM") as pps:

        # ---- constant / input loads -----------------------------------
        gt_t = pool.tile([128, FDH], U8, tag="gt")
        for gg in range(4):
            sl = slice(gg * FDH // 4, (gg + 1) * FDH // 4)
            nc.sync.dma_start(gt_t[:, sl], gt_d[:, sl])
        wn_t = pool.tile([128, 128], BF16, tag="wn")
        nc.sync.dma_start(wn_t[:, :], wn_d)
        wpos_t = pool.tile([128, 384], BF16, tag="wpos")
        nc.sync.dma_start(wpos_t[:, :], wpos_d)
        cw_t = pool.tile([4, 128], BF16, tag="cw")
        nc.sync.dma_start(cw_t[:, :], cw_d)
        crows_t = pool.tile([4, FDO], BF16, tag="crows")
        nc.sync.dma_start(crows_t[:, :], crows_d)

        # ---- all three class masks up front ---------------------------
        # class 1 chunked on DVE (starts as soon as gt chunk 0 lands);
        # classes 2-3 on the GpSimd engine, ahead of everything else in
        # its queue so they never wait behind softmax work
        fposs = []
        for ci, c in enumerate((1, 2, 3)):
            fpos = rot.tile([128, FDH], BF16, tag="fpos", bufs=3)
            if ci == 0:
                for ch in range(4):
                    sl = slice(ch * FDH // 4, (ch + 1) * FDH // 4)
                    nc.vector.tensor_scalar(fpos[:, sl], gt_t[:, sl],
                                            float(c), None, AL.is_equal)
            else:
                nc.gpsimd.tensor_scalar(fpos[:, :], gt_t[:, :], float(c),
                                        None, AL.is_equal)
            fposs.append(fpos)

        e_t = pool.tile([128, 4 * FDO], BF16, tag="et")
        den = pool.tile([128, FDO], BF16, tag="den")
        inv_t = pool.tile([128, FDO], BF16, tag="inv")
        HF = FDO // 2

        def emit_softmax():
            """bf16 softmax tail: exp on ACT, den/E-folds on GpSimd.
            Emitted after the first two neg x-passes so the ACT queue isn't
            clogged with exps when the PSUM->SBUF copies need it."""
            for cc in range(8):
                st = rot.tile([128, HF], F32, tag="stage", bufs=2)
                nc.sync.dma_start(st[:, :], net_d[:, cc * HF:(cc + 1) * HF])
                nc.scalar.activation(e_t[:, cc * HF:(cc + 1) * HF], st[:, :],
                                     AF.Exp)
            nc.gpsimd.tensor_tensor(den[:, :], e_t[:, 0:FDO],
                                    e_t[:, FDO:2 * FDO], AL.add)
            nc.gpsimd.tensor_tensor(den[:, :], den[:, :],
                                    e_t[:, 2 * FDO:3 * FDO], AL.add)
            nc.gpsimd.tensor_tensor(den[:, :], den[:, :],
                                    e_t[:, 3 * FDO:4 * FDO], AL.add)
            for hh in range(2):
                sl = slice(hh * HF, (hh + 1) * HF)
                lh = rot.tile([128, HF], F32, tag="stage", bufs=2)
                nc.scalar.activation(lh[:, :], den[:, sl], AF.Ln)
                nc.scalar.activation(inv_t[:, sl], lh[:, :], AF.Exp,
                                     scale=-1.0)
            # fold 1/den into classes 2-3 off-path; class 1's fold runs on
            # DVE right before phi (inv isn't ready any earlier anyway)
            for c in (2, 3):
                esl = e_t[:, c * FDO:(c + 1) * FDO]
                nc.gpsimd.tensor_tensor(esl, esl, inv_t[:, :], AL.mult)
            # gated weights e'_c * fpos_c for the separated pos-part
            # accumulation of classes 2-3 (e0 slot and den are both dead)
            for c, epb in ((2, eps[1]), (3, eps[2])):
                fv = fposs[c - 1][:, :].rearrange("p (y z) -> p y z", z=ZT)
                epv = epb[:, :].rearrange("p (y z) -> p y z", z=ZO)
                nc.gpsimd.tensor_tensor(epv[:, :, :],
                                        e_t[:, c * FDO:(c + 1) * FDO]
                                        .rearrange("p (y z) -> p y z", z=ZO),
                                        fv[:, 2:2 + Y, H:H + ZO], AL.mult)

        eps = {1: e_t[:, 0:FDO], 2: den}
        out_t = pool.tile([128, 6], F32, tag="out")
        scr = pool.tile([128, FDO], F32, tag="scr")
        c_lo, c_hi = 2 * ZT, 130 * ZT          # center y rows [2,130)
        sns, gs = {}, {}

        def neg_xpass(ci):
            """r = W @ fpos, is_le decode -> f1 {0,1,4,12}; then y/z passes
            down to nz, its DMA ship, and sqrt."""
            fpos = fposs[ci]
            f1 = rot.tile([128, FDH], BF16, tag="f1", bufs=1)
            off = 0
            for w in NEG_CHUNKS:
                ps = pps.tile([128, NEG_CHUNKS[0]], F32, tag="negps", bufs=2)
                for mm in range(0, w, 512):
                    mw = min(512, w - mm)
                    nc.tensor.matmul(ps[:, mm:mm + mw], wn_t[:, :],
                                     fpos[:, off + mm:off + mm + mw],
                                     start=True, stop=True)
                sx = rot.tile([128, NEG_CHUNKS[0]], BF16, tag="sx", bufs=2)
                nc.scalar.activation(sx[:, 0:w], ps[:, 0:w], AF.Copy)
                t1 = rot.tile([128, NEG_CHUNKS[0]], BF16, tag="t1", bufs=2)
                t2 = rot.tile([128, NEG_CHUNKS[0]], BF16, tag="t2", bufs=2)
                nc.vector.tensor_scalar(t1[:, 0:w], sx[:, 0:w], 10.0, None,
                                        AL.is_le)
                nc.vector.tensor_scalar(t2[:, 0:w], sx[:, 0:w], 2.0, 3.0,
                                        AL.is_le, AL.mult)
                # third indicator overwrites sx in place (last reader)
                nc.vector.tensor_scalar(sx[:, 0:w], sx[:, 0:w], 0.0, BIGN,
                                        AL.is_le, AL.mult)
                nc.vector.tensor_tensor(t1[:, 0:w], t1[:, 0:w], t2[:, 0:w],
                                        AL.add)
                nc.vector.tensor_tensor(f1[:, off:off + w], t1[:, 0:w],
                                        sx[:, 0:w], AL.add)
                off += w
            return f1

        def neg_yz(ci, f1):
            # y-pass: pair-mins, +d^2 biases (cheap 4x tensor_scalar), mins
            u1 = rot.tile([128, Y * ZT], BF16, tag="yu", bufs=2)
            u2 = rot.tile([128, Y * ZT], BF16, tag="yu", bufs=2)
            nc.vector.tensor_tensor(u1[:, :], f1[:, c_lo - ZT:c_hi - ZT],
                                    f1[:, c_lo + ZT:c_hi + ZT], AL.min)
            nc.vector.tensor_tensor(u2[:, :],
                                    f1[:, c_lo - 2 * ZT:c_hi - 2 * ZT],
                                    f1[:, c_lo + 2 * ZT:c_hi + 2 * ZT],
                                    AL.min)
            nc.vector.tensor_scalar(u1[:, :], u1[:, :], 1.0, None, AL.add)
            nc.vector.tensor_scalar(u2[:, :], u2[:, :], 4.0, None, AL.add)
            f2 = rot.tile([128, Y * ZT], BF16, tag="f2", bufs=2)
            nc.vector.tensor_tensor(f2[:, :], u1[:, :], f1[:, c_lo:c_hi],
                                    AL.min)
            nc.vector.tensor_tensor(f2[:, :], f2[:, :], u2[:, :], AL.min)
            # z-pass
            f2v = f2[:, :].rearrange("p (y z) -> p y z", z=ZT)
            m1 = rot.tile([128, FDO], BF16, tag="zm", bufs=2)
            m2 = rot.tile([128, FDO], BF16, tag="zm", bufs=2)
            m1v = m1[:, :].rearrange("p (y z) -> p y z", z=ZO)
            m2v = m2[:, :].rearrange("p (y z) -> p y z", z=ZO)
            nc.vector.tensor_tensor(m1v[:, :, :],
                                    f2v[:, :, H - 1:H - 1 + ZO],
                                    f2v[:, :, H + 1:H + 1 + ZO], AL.min)
            nc.vector.tensor_tensor(m2v[:, :, :],
                                    f2v[:, :, H - 2:H - 2 + ZO],
                                    f2v[:, :, H + 2:H + 2 + ZO], AL.min)
            nc.vector.tensor_scalar(m1[:, :], m1[:, :], 1.0, None, AL.add)
            nc.vector.tensor_scalar(m2[:, :], m2[:, :], 4.0, None, AL.add)
            m1v2 = m1[:, :].rearrange("p (y z) -> p y z", z=ZO)
            nc.vector.tensor_tensor(m1v2[:, :, :], m1v2[:, :, :],
                                    f2v[:, :, H:H + ZO], AL.min)
            nz = m2
            nc.vector.tensor_tensor(nz[:, :], m1[:, :], m2[:, :], AL.min)
            nc.sync.dma_start(nz_d[:, ci * FDO:(ci + 1) * FDO], nz[:, :])
            sn = rot.tile([128, FDO], BF16, tag="sn", bufs=2)
            nc.scalar.activation(sn[:, :], nz[:, :], AF.Sqrt)
            sns[ci] = sn

        def pos_stage(ci):
            """9-tap radix s = 256*c1 + 16*c2 + c3, decoded straight to the
            boundary-zeroed positive distance, gated by fpos."""
            fposv = fposs[ci][:, :].rearrange("p (y z) -> p y z", z=ZT)
            s_t = rot.tile([128, FDO], FP16, tag="sfp", bufs=2)
            y0 = 0
            while y0 < Y:
                ny = min(POS_YBLK, Y - y0)
                pw = ny * ZO
                psq = pps.tile([128, POS_YBLK * ZO], F32, tag="posps",
                               bufs=2)
                for ti, (dy, dz, bi) in enumerate(POS_TAPS):
                    ys = 2 + y0 + dy
                    nc.tensor.matmul(
                        psq[:, 0:pw], wpos_t[:, 128 * bi:128 * (bi + 1)],
                        fposv[:, ys:ys + ny, H + dz:H + dz + ZO],
                        start=(ti == 0), stop=False)
                nc.tensor.matmul(psq[:, 0:pw], cw_t[:, :],
                                 crows_t[:, y0 * ZO:y0 * ZO + pw],
                                 start=False, stop=True)
                nc.scalar.activation(s_t[:, y0 * ZO:y0 * ZO + pw],
                                     psq[:, 0:pw], AF.Copy)
                y0 += ny
            g1 = rot.tile([128, FDO], BF16, tag="pg", bufs=2)
            g2 = rot.tile([128, FDO], BF16, tag="pg", bufs=2)
            nc.vector.tensor_scalar(g1[:, :], s_t[:, :], 1536.0, SQ2,
                                    AL.is_ge, AL.mult)
            nc.vector.tensor_scalar(g2[:, :], s_t[:, :], 1728.0, SQ3 - SQ2,
                                    AL.is_ge, AL.mult)
            nc.vector.tensor_tensor(g1[:, :], g1[:, :], g2[:, :], AL.add)
            g1v = g1[:, :].rearrange("p (y z) -> p y z", z=ZO)
            if ci == 0:
                nc.vector.tensor_tensor(g1v[:, :, :], g1v[:, :, :],
                                        fposv[:, 2:2 + Y, H:H + ZO], AL.mult)
            if DEBUG_POS:
                nc.sync.dma_start(sdbg_d[:, ci * FDO:(ci + 1) * FDO],
                                  s_t[:, :])
                nc.sync.dma_start(gdbg_d[:, ci * FDO:(ci + 1) * FDO],
                                  g1[:, :])
            gs[ci] = g1

        def phi_accum(ci):
            # phi = sqrt(nz) - posd'; weight by e_c/den; row-reduce on ACT
            sn, g1 = sns[ci], gs[ci]
            esl = e_t[:, (ci + 1) * FDO:(ci + 2) * FDO]
            if ci == 0:
                nc.vector.tensor_tensor(sn[:, :], sn[:, :], g1[:, :],
                                        AL.subtract)
                nc.vector.tensor_tensor(sn[:, :], sn[:, :], esl, AL.mult)
                nc.scalar.activation(scr[:, :], sn[:, :], AF.Copy,
                                     accum_out=out_t[:, 0:1])
            else:
                # pos part: independent of nz/sqrt, overlaps the z-pass;
                # host subtracts this column
                nc.vector.tensor_tensor(g1[:, :], g1[:, :], eps[ci][:, :],
                                        AL.mult)
                nc.scalar.activation(scr[:, :], g1[:, :], AF.Copy,
                                     accum_out=out_t[:, 2 * ci:2 * ci + 1])
                if ci < 2:
                    nc.vector.tensor_tensor(sn[:, :], sn[:, :], esl, AL.mult)
                    nc.scalar.activation(scr[:, :], sn[:, :], AF.Copy,
                                         accum_out=out_t[:, 1:2])
                else:
                    for hh in range(2):
                        sl = slice(hh * HF, (hh + 1) * HF)
                        nc.vector.tensor_tensor(sn[:, sl], sn[:, sl],
                                                esl[:, sl], AL.mult)
                        nc.scalar.activation(scr[:, sl], sn[:, sl], AF.Copy,
                                             accum_out=out_t[:, 3 + hh:4 + hh])
                        if hh == 0:
                            nc.sync.dma_start(out_d[:, 0:3], out_t[:, 0:3])

        # software pipeline: class ci's pos stage and phi ride one
        # iteration behind its neg chain, so the next class's neg matmuls
        # are never queued behind a full pos-radix block on PE
        for ci in range(3):
            f1 = neg_xpass(ci)
            if ci >= 1:
                pos_stage(ci - 1)
            if ci == 1:
                emit_softmax()
            neg_yz(ci, f1)
            if ci == 1:
                # class 1's 1/den fold, on DVE just before its phi
                nc.vector.tensor_tensor(e_t[:, FDO:2 * FDO],
                                        e_t[:, FDO:2 * FDO], inv_t[:, :],
                                        AL.mult)
            if ci >= 1:
                phi_accum(ci - 1)
        pos_stage(2)
        phi_accum(2)

        nc.sync.dma_start(out_d[:, 3:6], out_t[:, 3:6])


_NC = None


def _get_nc():
    global _NC
    if _NC is None:
        nc = bacc.Bacc("TRN2", target_bir_lowering=False, debug=False,
                       num_devices=8)
        gt_d = nc.dram_tensor("gt", [128, FDH], U8, kind="ExternalInput").ap()
        net_d = nc.dram_tensor("net", [128, 4 * FDO], F32,
                               kind="ExternalInput").ap()
        wn_d = nc.dram_tensor("wn", [128, 128], BF16,
                              kind="ExternalInput").ap()
        wpos_d = nc.dram_tensor("wpos", [128, 384], BF16,
                                kind="ExternalInput").ap()
        cw_d = nc.dram_tensor("cw", [4, 128], BF16, kind="ExternalInput").ap()
        crows_d = nc.dram_tensor("crows", [4, FDO], BF16,
                                 kind="ExternalInput").ap()
        out_d = nc.dram_tensor("out", [128, 6], F32,
                               kind="ExternalOutput").ap()
        nz_d = nc.dram_tensor("nzv", [128, 3 * FDO], BF16,
                              kind="ExternalOutput").ap()
        sdbg_d = gdbg_d = None
        if DEBUG_POS:
            sdbg_d = nc.dram_tensor("sdbg", [128, 3 * FDO], FP16,
                                    kind="ExternalOutput").ap()
            gdbg_d = nc.dram_tensor("gdbg", [128, 3 * FDO], BF16,
                                    kind="ExternalOutput").ap()
        with TileContext(nc) as tc:
            _body(tc, gt_d, net_d, wn_d, wpos_d, cw_d, crows_d, out_d,
                  nz_d, sdbg_d, gdbg_d)
        nc.compile()
        _NC = nc
    return _NC


def _in_maps(net_output, gt):
    bf = ml_dtypes.bfloat16
    wn = (16 * np.eye(128) + 4 * (np.eye(128, k=1) + np.eye(128, k=-1))
          + np.eye(128, k=2) + np.eye(128, k=-2)).astype(bf)
    A = 256.0 * (np.eye(128, k=1) + np.eye(128, k=-1))
    Bw = 256.0 * np.eye(128) + 16.0 * (np.eye(128, k=1) + np.eye(128, k=-1))
    Cw = 16.0 * np.eye(128) + (np.eye(128, k=1) + np.eye(128, k=-1))
    wpos = np.concatenate([A, Bw, Cw], axis=1).astype(bf)
    edge = np.zeros(128); edge[[0, 127]] = 1.0
    cw = np.stack([256.0 * np.ones(128), np.ones(128),
                   256.0 * edge, edge]).astype(bf)

    gtu = np.asarray(gt)[:, 0].astype(np.uint8)
    gtz = np.pad(gtu, ((0, 0), (0, 0), (0, 0), (H, H)), constant_values=255)
    maps = []
    for core in range(8):
        b, zs = core // 4, core % 4
        z0 = zs * ZO
        sl = gtz[b, :, :, z0:z0 + ZT]                       # [128, 128, 28]
        gts = np.pad(sl, ((0, 0), (2, 2), (0, 0)), constant_values=255)
        nets = np.ascontiguousarray(
            np.transpose(net_output[b, :, :, :, z0:z0 + ZO], (1, 0, 2, 3)))
        # correction rows: out-of-volume tap reads count as foreground
        rowF = np.zeros((Y, ZO)); rowE = np.zeros((Y, ZO))
        yi = np.arange(Y)[:, None]
        zg = (z0 + np.arange(ZO))[None, :]
        for dy, dz, bi in POS_TAPS:
            outm = ((yi + dy < 0) | (yi + dy >= Y)
                    | (zg + dz < 0) | (zg + dz >= Z))
            rowF += outm * BAND_FULLW[bi]
            rowE += outm * BAND_EDGEW[bi]
        rowG = EW_SUM - rowE
        crows = np.stack([rowF.reshape(-1) // 256, rowF.reshape(-1) % 256,
                          rowG.reshape(-1) // 256, rowG.reshape(-1) % 256]
                         ).astype(bf)
        maps.append({
            "gt": gts.reshape(128, FDH),
            "net": nets.reshape(128, 4 * FDO).astype(np.float32),
            "wn": wn, "wpos": wpos, "cw": cw, "crows": crows,
        })
    return maps


def _pos_window_ok(gtu):
    """True iff no foreground voxel (any class 1..3) has its entire 3^3
    neighborhood foreground-of-the-same-class (i.e. pos2 <= 3 everywhere,
    out-of-volume treated as foreground)."""
    for c in range(1, C):
        m = gtu == c
        p = np.pad(m, ((0, 0), (1, 1), (1, 1), (1, 1)), constant_values=True)
        ex = p[:, :-2] & p[:, 1:-1] & p[:, 2:]
        ey = ex[:, :, :-2] & ex[:, :, 1:-1] & ex[:, :, 2:]
        ez = ey[:, :, :, :-2] & ey[:, :, :, 1:-1] & ey[:, :, :, 2:]
        if (m & ez).any():
            return False
    return True


def _fallback(net_output, gt):
    """Exact host computation (safety net if the windowed-EDT verification
    fails)."""
    from scipy import ndimage
    net = np.asarray(net_output, np.float64)
    g = np.asarray(gt)[:, 0]
    e = np.exp(net - net.max(axis=1, keepdims=True))
    probs = e / e.sum(axis=1, keepdims=True)
    tot = 0.0
    for b in range(B):
        for c in range(1, C):
            m = g[b] == c
            if not m.any():
                continue
            pos = ndimage.distance_transform_edt(m)
            neg = ndimage.distance_transform_edt(~m)
            er = ndimage.binary_erosion(
                m, structure=ndimage.generate_binary_structure(3, 1),
                border_value=1)
            phi = np.where(m & ~er, 0.0, neg - pos)
            tot += float((probs[b, c] * phi).sum())
    return np.float32(tot / NVOX)


def kernel(net_output, gt, _spmd_result=[None]):
    nc = _get_nc()
    res = bass_utils.run_bass_kernel_spmd(nc, _in_maps(net_output, gt),
                                          core_ids=list(range(8)))
    _spmd_result[0] = res
    total, ok = 0.0, True
    for r in res.results:
        o = np.asarray(r["out"], np.float64)
        # cols: 0 class1-phi, 1 class2-neg, 3+4 class3-neg halves,
        #       2 class2-pos, 5 class3-pos (subtracted)
        total += o[:, [0, 1, 3, 4]].sum() - o[:, [2, 5]].sum()
        nv = np.asarray(r["nzv"]).astype(np.float32)
        ok &= bool(nv.max() <= 8.5)
    ok = ok and _pos_window_ok(np.asarray(gt)[:, 0])
    if not ok:
        return _fallback(net_output, gt)
    return np.float32(total / NVOX)



# revision 7
# speedup vs baseline: 1.2987x; 1.2987x over previous
"""Boundary-distance loss (BDLoss) on 8 Trainium2 NeuronCores.

Math (matches the reference):
  probs = softmax(net_output, axis=1)
  onehot_c = (gt == c)
  posdis = EDT(onehot_c), negdis = EDT(~onehot_c)
  phi = where(inner_boundary, 0, negdis - posdis), zeroed if class absent
  out  = mean(probs[:, 1:] * phi[:, 1:])

Algorithm (exponential-weight separable convolution):
  * NEG field: E = conv3d(fpos, beta^-d^2) over the 5^3 box with beta = 2^8.
    Since all weights are powers of two and counts per distance-shell are
    < 32, the EXPONENT FIELD of E encodes m = min d^2 exactly:
    x := (bits(E) + 512) >> 10 == 16 - m (x == 0 for "nothing in box").
    The conv is separable: the x-taps ride in banded 128x128 matmul weights,
    the y-taps are 5 shifted matmul passes accumulated in PSUM (all on PE),
    and the z-taps are 4 pair-adds + 2 scales on DVE.  sqrt(m) is a single
    ACT pass: sqrt(-1*x + 16).
  * POS field: posE = conv3d(fneg, beta^-d^2) over the 3^3 box (fneg = "bg
    for class c", with OOV/pads forced to 0 so out-of-volume never counts
    as background).  Then the boundary-zeroed positive distance is
    posd = sqrt2*[posE < 2^-9] + (sqrt3-sqrt2)*[posE < 2^-17]
    (thresholds = "no bg at d^2<=1 / <=2"), auto-zero on bg voxels because
    the center tap makes posE >= 1 there.
  * phi = sqrt(m) - posd; result = sum(probs_c * phi) via a DVE
    tensor_scalar with accum_out per class; host sums across cores.
  * Verification: x-fields are shipped; min(x) >= 8 iff the windowed box
    EDT is exact (neg^2 <= 8 everywhere); pos exactness (pos^2 <= 3) is
    checked with numpy erosion.  On failure -> exact scipy fallback.

Sharding: core = (b, z-slab): b = core//4, z0 = 24*(core%4).  gt is sent as
uint16 with a 2-plane z halo and 2-column y pad of value 255.
"""

import numpy as np
import ml_dtypes

import concourse.bacc as bacc
import concourse.mybir as mybir
from concourse.tile import TileContext
from concourse import bass_utils

F32 = mybir.dt.float32
BF16 = mybir.dt.bfloat16
U16 = mybir.dt.uint16
AL = mybir.AluOpType
AF = mybir.ActivationFunctionType

B, C, X, Y, Z = 2, 4, 128, 128, 96
ZO = 24            # output z-planes per core
H = 2              # z halo
ZT = ZO + 2 * H    # 28 z-planes on chip
YP = Y + 4         # y padded to 132
FDH = YP * ZT      # 3696 cols of the padded mask tile
FDO = Y * ZO       # 3072 cols of a dense output tile
NVOX = B * (C - 1) * X * Y * Z
SQ2 = float(np.sqrt(2.0))
SQ3 = float(np.sqrt(3.0))
LB = 2.0 ** -8     # conv base beta^-1
T1 = 2.0 ** -9     # pos threshold: no bg at d^2 <= 1
T2 = 2.0 ** -17    # pos threshold: no bg at d^2 <= 2
CW = 3584          # conv output cols: y in [2,130) x z in [0,28)
C_LO = 2 * ZT      # first output col
CHUNK = 1024       # psum chunk (2 banks)
SUB = 512          # matmul sub-block (1 bank)

NEG_PASSES = [(2, -2), (2, 2), (1, -1), (1, 1), (0, 0)]   # (wn band idx, dy)
POS_PASSES = [(1, -1), (1, 1), (0, 0)]                    # (wp band idx, dy)


def _body(tc, gt_d, net_d, wn_d, wp_d, out_d, x_d):
    nc = tc.nc
    with tc.tile_pool(name="main", bufs=1) as pool, \
         tc.tile_pool(name="rot", bufs=2) as rot, \
         tc.tile_pool(name="nps", bufs=2, space="PSUM") as nps, \
         tc.tile_pool(name="pps", bufs=2, space="PSUM") as pps:

        # ---- input loads ---------------------------------------------
        gt16 = pool.tile([128, FDH], U16, tag="gt")
        for gg in range(4):
            sl = slice(gg * FDH // 4, (gg + 1) * FDH // 4)
            nc.sync.dma_start(gt16[:, sl], gt_d[:, sl])
        wn_t = pool.tile([128, 384], BF16, tag="wn")
        nc.sync.dma_start(wn_t[:, :], wn_d)
        wp_t = pool.tile([128, 256], BF16, tag="wp")
        nc.sync.dma_start(wp_t[:, :], wp_d)
        b16 = pool.tile([128, 1], F32, tag="b16")
        nc.vector.memset(b16[:, :], 16.0)

        # ---- softmax exps (ACT queue head; runs under PE conv work) --
        e_t = pool.tile([128, 4 * FDO], BF16, tag="et")
        HF = FDO // 2
        for cc in range(8):
            st = rot.tile([128, HF], F32, tag="stage", bufs=2)
            nc.sync.dma_start(st[:, :], net_d[:, cc * HF:(cc + 1) * HF])
            nc.scalar.activation(e_t[:, cc * HF:(cc + 1) * HF], st[:, :],
                                 AF.Exp)

        # ---- masks (DVE), built one class ahead of their convs -------
        gtv = gt16[:, :].rearrange("p (y z) -> p y z", z=ZT)
        vh = pool.tile([128, 4 * YP], BF16, tag="vh")
        vhv = vh[:, :].rearrange("p (y z) -> p y z", z=4)
        nc.vector.tensor_scalar(vhv[:, :, 0:2], gtv[:, :, 0:2], 3, None,
                                AL.is_le)
        nc.vector.tensor_scalar(vhv[:, :, 2:4], gtv[:, :, ZT - 2:ZT], 3,
                                None, AL.is_le)

        def build_masks(c):
            fpos = rot.tile([128, FDH], BF16, tag="fpos", bufs=2)
            nc.vector.tensor_scalar(fpos[:, :], gt16[:, :], c, None,
                                    AL.is_equal)
            fneg = rot.tile([128, FDH], BF16, tag="fneg", bufs=2)
            nc.vector.tensor_scalar(fneg[:, :], gt16[:, :], c, None,
                                    AL.not_equal)
            # y pads -> 0 (OOV is never background)
            nc.gpsimd.memset(fneg[:, 0:2 * ZT], 0.0)
            nc.gpsimd.memset(fneg[:, 130 * ZT:132 * ZT], 0.0)
            # z halo -> V - fpos (0 on OOV pad, unchanged on real data)
            fnv = fneg[:, :].rearrange("p (y z) -> p y z", z=ZT)
            fpv = fpos[:, :].rearrange("p (y z) -> p y z", z=ZT)
            nc.vector.tensor_tensor(fnv[:, :, 0:2], vhv[:, :, 0:2],
                                    fpv[:, :, 0:2], AL.subtract)
            nc.vector.tensor_tensor(fnv[:, :, ZT - 2:ZT], vhv[:, :, 2:4],
                                    fpv[:, :, ZT - 2:ZT], AL.subtract)
            return fpos, fneg

        den = pool.tile([128, FDO], BF16, tag="den")
        inv_t = pool.tile([128, FDO], BF16, tag="inv")
        out_t = pool.tile([128, 4], F32, tag="out")
        scr = pool.tile([128, FDO], BF16, tag="scr")
        xzs, gs = {}, {}
        masks = [build_masks(1)]

        def conv_xy(mask, passes, wmat, tag, ppool):
            """x-band (x) y-shift (xy) conv passes -> SBUF bf16 [128, CW]."""
            sb = rot.tile([128, CW], BF16, tag=tag, bufs=2)
            off = 0
            while off < CW:
                w = min(CHUNK, CW - off)
                ps = ppool.tile([128, CHUNK], F32, tag=tag + "ps", bufs=2)
                for pi, (wi, dy) in enumerate(passes):
                    first, last = pi == 0, pi == len(passes) - 1
                    for mm in range(0, w, SUB):
                        mw = min(SUB, w - mm)
                        a = C_LO + dy * ZT + off + mm
                        nc.tensor.matmul(ps[:, mm:mm + mw],
                                         wmat[:, 128 * wi:128 * (wi + 1)],
                                         mask[:, a:a + mw],
                                         start=first, stop=last)
                nc.scalar.activation(sb[:, off:off + w], ps[:, 0:w], AF.Copy)
                off += w
            return sb

        def zconv5(sb):
            """5-tap z-conv on DVE -> flat [128, FDO] bf16 (= tile A)."""
            v = sb[:, :].rearrange("p (y z) -> p y z", z=ZT)
            a = rot.tile([128, FDO], BF16, tag="A", bufs=2)
            b = rot.tile([128, FDO], BF16, tag="Bt", bufs=2)
            av = a[:, :].rearrange("p (y z) -> p y z", z=ZO)
            bv = b[:, :].rearrange("p (y z) -> p y z", z=ZO)
            nc.vector.tensor_tensor(av[:, :, :], v[:, :, 1:1 + ZO],
                                    v[:, :, 3:3 + ZO], AL.add)
            nc.vector.tensor_tensor(bv[:, :, :], v[:, :, 0:ZO],
                                    v[:, :, 4:4 + ZO], AL.add)
            nc.vector.tensor_scalar(a[:, :], a[:, :], LB, None, AL.mult)
            nc.vector.tensor_scalar(b[:, :], b[:, :], LB ** 4, None, AL.mult)
            nc.vector.tensor_tensor(av[:, :, :], v[:, :, 2:2 + ZO],
                                    av[:, :, :], AL.add)
            nc.vector.tensor_tensor(a[:, :], a[:, :], b[:, :], AL.add)
            return a, b

        def zconv3(sb):
            """3-tap z-conv on DVE -> flat [128, FDO] bf16."""
            v = sb[:, :].rearrange("p (y z) -> p y z", z=ZT)
            a = rot.tile([128, FDO], BF16, tag="Bt", bufs=2)
            av = a[:, :].rearrange("p (y z) -> p y z", z=ZO)
            nc.vector.tensor_tensor(av[:, :, :], v[:, :, 1:1 + ZO],
                                    v[:, :, 3:3 + ZO], AL.add)
            nc.vector.tensor_scalar(a[:, :], a[:, :], LB, None, AL.mult)
            nc.vector.tensor_tensor(av[:, :, :], v[:, :, 2:2 + ZO],
                                    av[:, :, :], AL.add)
            return a

        for ci in range(3):
            fpos, fneg = masks[ci]
            exy = conv_xy(fpos, NEG_PASSES, wn_t, "exy", nps)
            cxy = conv_xy(fneg, POS_PASSES, wp_t, "cxy", pps)
            if ci < 2:
                masks.append(build_masks(ci + 2))
            if ci == 0:
                # den + 1/den + probability folds (Pool + ACT, off-path)
                nc.gpsimd.tensor_tensor(den[:, :], e_t[:, 0:FDO],
                                        e_t[:, FDO:2 * FDO], AL.add)
                nc.gpsimd.tensor_tensor(den[:, :], den[:, :],
                                        e_t[:, 2 * FDO:3 * FDO], AL.add)
                nc.gpsimd.tensor_tensor(den[:, :], den[:, :],
                                        e_t[:, 3 * FDO:4 * FDO], AL.add)
                for hh in range(2):
                    sl = slice(hh * HF, (hh + 1) * HF)
                    lh = rot.tile([128, HF], F32, tag="stage", bufs=2)
                    nc.scalar.activation(lh[:, :], den[:, sl], AF.Ln)
                    nc.scalar.activation(inv_t[:, sl], lh[:, :], AF.Exp,
                                         scale=-1.0)
                for c in (1, 2, 3):
                    esl = e_t[:, c * FDO:(c + 1) * FDO]
                    nc.gpsimd.tensor_tensor(esl, esl, inv_t[:, :], AL.mult)
            # neg z + exponent decode -> x = 16 - m (u16)
            ez, bt = zconv5(exy)
            xz = rot.tile([128, FDO], U16, tag="xz", bufs=3)
            nc.vector.tensor_scalar(bt[:, :].bitcast(U16),
                                    ez[:, :].bitcast(U16), 512, None, AL.add)
            nc.vector.tensor_scalar(xz[:, :], bt[:, :].bitcast(U16), 10,
                                    None, AL.logical_shift_right)
            nc.sync.dma_start(x_d[:, ci * FDO:(ci + 1) * FDO], xz[:, :])
            xzs[ci] = xz
            # pos z + thresholds -> g = posd'
            pe = zconv3(cxy)
            g1 = rot.tile([128, FDO], BF16, tag="g", bufs=3)
            g2 = rot.tile([128, FDO], BF16, tag="Bt", bufs=2)
            nc.vector.tensor_scalar(g1[:, :], pe[:, :], T1, SQ2, AL.is_lt,
                                    AL.mult)
            nc.vector.tensor_scalar(g2[:, :], pe[:, :], T2, SQ3 - SQ2,
                                    AL.is_lt, AL.mult)
            nc.vector.tensor_tensor(g1[:, :], g1[:, :], g2[:, :], AL.add)
            gs[ci] = g1

        # ---- sqrts batched (one act-table switch), then tails --------
        for ci in range(3):
            sn = rot.tile([128, FDO], BF16, tag="sn", bufs=2)
            nc.scalar.activation(sn[:, :], xzs[ci][:, :], AF.Sqrt,
                                 bias=b16[:, 0:1], scale=-1.0)
            g = gs[ci]
            nc.vector.tensor_tensor(g[:, :], sn[:, :], g[:, :],
                                    AL.subtract)
            nc.vector.tensor_tensor(g[:, :], g[:, :],
                                    e_t[:, (ci + 1) * FDO:(ci + 2) * FDO],
                                    AL.mult)
            nc.vector.tensor_scalar(scr[:, :], g[:, :], 1.0, 0.0, AL.mult,
                                    AL.add, accum_out=out_t[:, ci:ci + 1])
        nc.vector.memset(out_t[:, 3:4], 0.0)
        nc.sync.dma_start(out_d[:, :], out_t[:, :])


_NC = None


def _get_nc():
    global _NC
    if _NC is None:
        nc = bacc.Bacc("TRN2", target_bir_lowering=False, debug=False,
                       num_devices=8)
        gt_d = nc.dram_tensor("gt", [128, FDH], U16,
                              kind="ExternalInput").ap()
        net_d = nc.dram_tensor("net", [128, 4 * FDO], F32,
                               kind="ExternalInput").ap()
        wn_d = nc.dram_tensor("wn", [128, 384], BF16,
                              kind="ExternalInput").ap()
        wp_d = nc.dram_tensor("wp", [128, 256], BF16,
                              kind="ExternalInput").ap()
        out_d = nc.dram_tensor("out", [128, 4], F32,
                               kind="ExternalOutput").ap()
        x_d = nc.dram_tensor("xs", [128, 3 * FDO], U16,
                             kind="ExternalOutput").ap()
        with TileContext(nc) as tc:
            _body(tc, gt_d, net_d, wn_d, wp_d, out_d, x_d)
        nc.compile()
        _NC = nc
    return _NC


def _in_maps(net_output, gt):
    bf = ml_dtypes.bfloat16
    b0 = (np.eye(128) + LB * (np.eye(128, k=1) + np.eye(128, k=-1))
          + LB ** 4 * (np.eye(128, k=2) + np.eye(128, k=-2)))
    wn = np.concatenate([b0, LB * b0, LB ** 4 * b0], axis=1).astype(bf)
    p0 = np.eye(128) + LB * (np.eye(128, k=1) + np.eye(128, k=-1))
    wp = np.concatenate([p0, LB * p0], axis=1).astype(bf)

    gtu = np.asarray(gt)[:, 0].astype(np.uint16)
    gtz = np.pad(gtu, ((0, 0), (0, 0), (0, 0), (H, H)), constant_values=255)
    maps = []
    for core in range(8):
        b, zs = core // 4, core % 4
        z0 = zs * ZO
        sl = gtz[b, :, :, z0:z0 + ZT]                       # [128, 128, 28]
        gts = np.pad(sl, ((0, 0), (2, 2), (0, 0)), constant_values=255)
        nets = np.ascontiguousarray(
            np.transpose(net_output[b, :, :, :, z0:z0 + ZO], (1, 0, 2, 3)))
        maps.append({
            "gt": gts.reshape(128, FDH),
            "net": nets.reshape(128, 4 * FDO).astype(np.float32),
            "wn": wn, "wp": wp,
        })
    return maps


def _pos_window_ok(gtu):
    """True iff no foreground voxel (any class 1..3) has its entire 3^3
    neighborhood foreground-of-the-same-class (pos2 <= 3 everywhere,
    out-of-volume treated as foreground)."""
    for c in range(1, C):
        m = gtu == c
        p = np.pad(m, ((0, 0), (1, 1), (1, 1), (1, 1)), constant_values=True)
        ex = p[:, :-2] & p[:, 1:-1] & p[:, 2:]
        ey = ex[:, :, :-2] & ex[:, :, 1:-1] & ex[:, :, 2:]
        ez = ey[:, :, :, :-2] & ey[:, :, :, 1:-1] & ey[:, :, :, 2:]
        if (m & ez).any():
            return False
    return True


def _fallback(net_output, gt):
    """Exact host computation (safety net if windowed-EDT verification
    fails)."""
    from scipy import ndimage
    net = np.asarray(net_output, np.float64)
    g = np.asarray(gt)[:, 0]
    e = np.exp(net - net.max(axis=1, keepdims=True))
    probs = e / e.sum(axis=1, keepdims=True)
    tot = 0.0
    for b in range(B):
        for c in range(1, C):
            m = g[b] == c
            if not m.any():
                continue
            pos = ndimage.distance_transform_edt(m)
            neg = ndimage.distance_transform_edt(~m)
            er = ndimage.binary_erosion(
                m, structure=ndimage.generate_binary_structure(3, 1),
                border_value=1)
            phi = np.where(m & ~er, 0.0, neg - pos)
            tot += float((probs[b, c] * phi).sum())
    return np.float32(tot / NVOX)


def kernel(net_output, gt, _spmd_result=[None]):
    nc = _get_nc()
    res = bass_utils.run_bass_kernel_spmd(nc, _in_maps(net_output, gt),
                                          core_ids=list(range(8)))
    _spmd_result[0] = res
    total, ok = 0.0, True
    for r in res.results:
        o = np.asarray(r["out"], np.float64)
        total += o[:, 0:3].sum()
        xs = np.asarray(r["xs"])
        ok &= bool(xs.min() >= 8)       # x = 16 - m; need m <= 8 everywhere
    ok = ok and _pos_window_ok(np.asarray(gt)[:, 0])
    if not ok:
        return _fallback(net_output, gt)
    return np.float32(total / NVOX)


# revision 12
# speedup vs baseline: 1.3000x; 1.0010x over previous
"""Boundary-distance loss (BDLoss) on 8 Trainium2 NeuronCores.

Math (matches the reference):
  probs = softmax(net_output, axis=1)
  onehot_c = (gt == c)
  posdis = EDT(onehot_c), negdis = EDT(~onehot_c)
  phi = where(inner_boundary, 0, negdis - posdis), zeroed if class absent
  out  = mean(probs[:, 1:] * phi[:, 1:])

Algorithm (exponential-weight separable convolution):
  * NEG field: E = conv3d(fpos, beta^-d^2) over the 5^3 box with beta = 2^8.
    Since all weights are powers of two and counts per distance-shell are
    < 32, the EXPONENT FIELD of E encodes m = min d^2 exactly:
    x := (bits(E) + 512) >> 10 == 16 - m (x == 0 for "nothing in box").
    The conv is separable: the x-taps ride in banded 128x128 matmul weights,
    the y-taps are 5 shifted matmul passes accumulated in PSUM (all on PE),
    and the z-taps are 4 pair-adds + 2 scales on DVE.  sqrt(m) is a single
    ACT pass: sqrt(-1*x + 16).
  * POS field: posE = conv3d(fneg, beta^-d^2) over the 3^3 box (fneg = "bg
    for class c", with OOV/pads forced to 0 so out-of-volume never counts
    as background).  Then the boundary-zeroed positive distance is
    posd = sqrt2*[posE < 2^-9] + (sqrt3-sqrt2)*[posE < 2^-17]
    (thresholds = "no bg at d^2<=1 / <=2"), auto-zero on bg voxels because
    the center tap makes posE >= 1 there.
  * phi = sqrt(m) - posd; result = sum(probs_c * phi) via a DVE
    tensor_scalar with accum_out per class; host sums across cores.
  * Verification: x-fields are shipped; min(x) >= 8 iff the windowed box
    EDT is exact (neg^2 <= 8 everywhere); pos exactness (pos^2 <= 3) is
    checked with numpy erosion.  On failure -> exact scipy fallback.

Sharding: core = (b, z-slab): b = core//4, z0 = 24*(core%4).  gt is sent as
uint16 with a 2-plane z halo and 2-column y pad of value 255.
"""

import numpy as np
import ml_dtypes

import concourse.bacc as bacc
import concourse.mybir as mybir
from concourse.tile import TileContext
from concourse import bass_utils

F32 = mybir.dt.float32
BF16 = mybir.dt.bfloat16
U16 = mybir.dt.uint16
AL = mybir.AluOpType
AF = mybir.ActivationFunctionType

B, C, X, Y, Z = 2, 4, 128, 128, 96
ZO = 24            # output z-planes per core
H = 2              # z halo
ZT = ZO + 2 * H    # 28 z-planes on chip
YP = Y + 4         # y padded to 132
FDH = YP * ZT      # 3696 cols of the padded mask tile
FDO = Y * ZO       # 3072 cols of a dense output tile
NVOX = B * (C - 1) * X * Y * Z
SQ2 = float(np.sqrt(2.0))
SQ3 = float(np.sqrt(3.0))
LB = 2.0 ** -8     # conv base beta^-1
T1 = 2.0 ** -9     # pos threshold: no bg at d^2 <= 1
T2 = 2.0 ** -17    # pos threshold: no bg at d^2 <= 2
CW = 3584          # conv output cols: y in [2,130) x z in [0,28)
C_LO = 2 * ZT      # first output col
CHUNK = 1024       # psum chunk (2 banks)
SUB = 512          # matmul sub-block (1 bank)

NEG_PASSES = [(2, -2), (2, 2), (1, -1), (1, 1), (0, 0)]   # (wn band idx, dy)
POS_PASSES = [(1, -1), (1, 1), (0, 0)]                    # (wp band idx, dy)


def _body(tc, gt_d, net_d, wn_d, wp_d, out_d, x_d):
    nc = tc.nc
    with tc.tile_pool(name="main", bufs=1) as pool, \
         tc.tile_pool(name="rot", bufs=2) as rot, \
         tc.tile_pool(name="nps", bufs=2, space="PSUM") as nps, \
         tc.tile_pool(name="pps", bufs=2, space="PSUM") as pps:

        # ---- input loads ---------------------------------------------
        gt16 = pool.tile([128, FDH], U16, tag="gt")
        for gg in range(4):
            sl = slice(gg * FDH // 4, (gg + 1) * FDH // 4)
            nc.sync.dma_start(gt16[:, sl], gt_d[:, sl])
        wn_t = pool.tile([128, 384], BF16, tag="wn")
        nc.sync.dma_start(wn_t[:, :], wn_d)
        wp_t = pool.tile([128, 256], BF16, tag="wp")
        nc.sync.dma_start(wp_t[:, :], wp_d)
        b16 = pool.tile([128, 1], F32, tag="b16")
        nc.vector.memset(b16[:, :], 16.0)

        # ---- softmax exps: 2 now, 6 after class-1 copy emission ------
        e_t = pool.tile([128, 4 * FDO], BF16, tag="et")
        HF = FDO // 2

        def emit_exps(rng):
            for cc in rng:
                st = rot.tile([128, HF], F32, tag="stage", bufs=2)
                nc.sync.dma_start(st[:, :], net_d[:, cc * HF:(cc + 1) * HF])
                nc.scalar.activation(e_t[:, cc * HF:(cc + 1) * HF], st[:, :],
                                     AF.Exp)

        emit_exps(range(2))

        # ---- masks (DVE), built one class ahead of their convs -------
        gtv = gt16[:, :].rearrange("p (y z) -> p y z", z=ZT)
        vh = pool.tile([128, 4 * YP], BF16, tag="vh")
        vhv = vh[:, :].rearrange("p (y z) -> p y z", z=4)
        nc.vector.tensor_scalar(vhv[:, :, 0:2], gtv[:, :, 0:2], 3, None,
                                AL.is_le)
        nc.vector.tensor_scalar(vhv[:, :, 2:4], gtv[:, :, ZT - 2:ZT], 3,
                                None, AL.is_le)

        def build_masks(c, nch=1):
            fpos = rot.tile([128, FDH], BF16, tag="fpos", bufs=3)
            fneg = rot.tile([128, FDH], BF16, tag="fneg", bufs=3)
            for g in range(nch):
                sl = slice(g * FDH // nch, (g + 1) * FDH // nch)
                nc.vector.tensor_scalar(fpos[:, sl], gt16[:, sl], c, None,
                                        AL.is_equal)
                nc.vector.tensor_scalar(fneg[:, sl], gt16[:, sl], c, None,
                                        AL.not_equal)
            # y pads -> 0 (OOV is never background)
            nc.gpsimd.memset(fneg[:, 0:2 * ZT], 0.0)
            nc.gpsimd.memset(fneg[:, 130 * ZT:132 * ZT], 0.0)
            # z halo -> V - fpos (0 on OOV pad, unchanged on real data)
            fnv = fneg[:, :].rearrange("p (y z) -> p y z", z=ZT)
            fpv = fpos[:, :].rearrange("p (y z) -> p y z", z=ZT)
            nc.vector.tensor_tensor(fnv[:, :, 0:2], vhv[:, :, 0:2],
                                    fpv[:, :, 0:2], AL.subtract)
            nc.vector.tensor_tensor(fnv[:, :, ZT - 2:ZT], vhv[:, :, 2:4],
                                    fpv[:, :, ZT - 2:ZT], AL.subtract)
            return fpos, fneg

        den = pool.tile([128, FDO], BF16, tag="den")
        inv_t = pool.tile([128, FDO], BF16, tag="inv")
        out_t = pool.tile([128, 4], F32, tag="out")
        scr = e_t[:, 0:FDO]        # e0 slot is dead once den exists
        xzs, gs = {}, {}
        masks = [build_masks(1, nch=4)]

        def conv_xy(mask, passes, wmat, tag, ppool):
            """x-band (x) y-shift (xy) conv passes -> SBUF bf16 [128, CW]."""
            sb = rot.tile([128, CW], BF16, tag=tag, bufs=2)
            off = 0
            while off < CW:
                w = min(CHUNK, CW - off)
                ps = ppool.tile([128, CHUNK], F32, tag=tag + "ps", bufs=2)
                for pi, (wi, dy) in enumerate(passes):
                    first, last = pi == 0, pi == len(passes) - 1
                    for mm in range(0, w, SUB):
                        mw = min(SUB, w - mm)
                        a = C_LO + dy * ZT + off + mm
                        nc.tensor.matmul(ps[:, mm:mm + mw],
                                         wmat[:, 128 * wi:128 * (wi + 1)],
                                         mask[:, a:a + mw],
                                         start=first, stop=last)
                nc.scalar.activation(sb[:, off:off + w], ps[:, 0:w], AF.Copy)
                off += w
            return sb

        YH = Y // 2

        def zconv5(sb):
            """5-tap z-conv on DVE (y-halves) -> flat [128, FDO] bf16."""
            v = sb[:, :].rearrange("p (y z) -> p y z", z=ZT)
            a = rot.tile([128, FDO], BF16, tag="A", bufs=2)
            b = rot.tile([128, FDO], BF16, tag="Bt", bufs=2)
            av = a[:, :].rearrange("p (y z) -> p y z", z=ZO)
            bv = b[:, :].rearrange("p (y z) -> p y z", z=ZO)
            for y0 in (0, YH):
                ys = slice(y0, y0 + YH)
                nc.vector.tensor_tensor(av[:, ys, :], v[:, ys, 1:1 + ZO],
                                        v[:, ys, 3:3 + ZO], AL.add)
                nc.vector.tensor_tensor(bv[:, ys, :], v[:, ys, 0:ZO],
                                        v[:, ys, 4:4 + ZO], AL.add)
                nc.vector.tensor_scalar(av[:, ys, :], av[:, ys, :], LB,
                                        None, AL.mult)
                nc.vector.tensor_scalar(bv[:, ys, :], bv[:, ys, :], LB ** 4,
                                        None, AL.mult)
                nc.vector.tensor_tensor(av[:, ys, :], v[:, ys, 2:2 + ZO],
                                        av[:, ys, :], AL.add)
                nc.vector.tensor_tensor(av[:, ys, :], av[:, ys, :],
                                        bv[:, ys, :], AL.add)
            return a, b

        def zconv3(sb):
            """3-tap z-conv on DVE (y-halves) -> flat [128, FDO] bf16."""
            v = sb[:, :].rearrange("p (y z) -> p y z", z=ZT)
            a = rot.tile([128, FDO], BF16, tag="Bt", bufs=2)
            av = a[:, :].rearrange("p (y z) -> p y z", z=ZO)
            for y0 in (0, YH):
                ys = slice(y0, y0 + YH)
                nc.vector.tensor_tensor(av[:, ys, :], v[:, ys, 1:1 + ZO],
                                        v[:, ys, 3:3 + ZO], AL.add)
                nc.vector.tensor_scalar(av[:, ys, :], av[:, ys, :], LB,
                                        None, AL.mult)
                nc.vector.tensor_tensor(av[:, ys, :], v[:, ys, 2:2 + ZO],
                                        av[:, ys, :], AL.add)
            return a

        for ci in range(3):
            fpos, fneg = masks[ci]
            exy = conv_xy(fpos, NEG_PASSES, wn_t, "exy", nps)
            cxy = conv_xy(fneg, POS_PASSES, wp_t, "cxy", pps)
            if ci < 2:
                masks.append(build_masks(ci + 2))
            if ci == 0:
                emit_exps(range(2, 8))
                # den + 1/den + probability folds (Pool + ACT, off-path)
                nc.gpsimd.tensor_tensor(den[:, :], e_t[:, 0:FDO],
                                        e_t[:, FDO:2 * FDO], AL.add)
                nc.gpsimd.tensor_tensor(den[:, :], den[:, :],
                                        e_t[:, 2 * FDO:3 * FDO], AL.add)
                nc.gpsimd.tensor_tensor(den[:, :], den[:, :],
                                        e_t[:, 3 * FDO:4 * FDO], AL.add)
                for hh in range(2):
                    sl = slice(hh * HF, (hh + 1) * HF)
                    lh = rot.tile([128, HF], F32, tag="stage", bufs=2)
                    nc.scalar.activation(lh[:, :], den[:, sl], AF.Ln)
                    nc.scalar.activation(inv_t[:, sl], lh[:, :], AF.Exp,
                                         scale=-1.0)
                for c in (1, 2, 3):
                    esl = e_t[:, c * FDO:(c + 1) * FDO]
                    nc.gpsimd.tensor_tensor(esl, esl, inv_t[:, :], AL.mult)
            # neg z + exponent decode -> x = 16 - m (u16)
            ez, bt = zconv5(exy)
            xz = rot.tile([128, FDO], U16, tag="xz", bufs=3)
            nc.vector.tensor_scalar(bt[:, :].bitcast(U16),
                                    ez[:, :].bitcast(U16), 512, None, AL.add)
            nc.vector.tensor_scalar(xz[:, :], bt[:, :].bitcast(U16), 10,
                                    None, AL.logical_shift_right)
            nc.sync.dma_start(x_d[:, ci * FDO:(ci + 1) * FDO], xz[:, :])
            xzs[ci] = xz
            # pos z + thresholds -> g = posd'
            pe = zconv3(cxy)
            g1 = rot.tile([128, FDO], BF16, tag="g", bufs=3)
            g2 = rot.tile([128, FDO], BF16, tag="Bt", bufs=2)
            nc.vector.tensor_scalar(g1[:, :], pe[:, :], T1, SQ2, AL.is_lt,
                                    AL.mult)
            nc.vector.tensor_scalar(g2[:, :], pe[:, :], T2, SQ3 - SQ2,
                                    AL.is_lt, AL.mult)
            nc.vector.tensor_tensor(g1[:, :], g1[:, :], g2[:, :], AL.add)
            gs[ci] = g1

        # ---- sqrts batched (one act-table switch), then tails --------
        for ci in range(3):
            sn = rot.tile([128, FDO], BF16, tag="sn", bufs=2)
            nc.scalar.activation(sn[:, :], xzs[ci][:, :], AF.Sqrt,
                                 bias=b16[:, 0:1], scale=-1.0)
            g = gs[ci]
            nc.vector.tensor_tensor(g[:, :], sn[:, :], g[:, :],
                                    AL.subtract)
            nc.vector.tensor_tensor(g[:, :], g[:, :],
                                    e_t[:, (ci + 1) * FDO:(ci + 2) * FDO],
                                    AL.mult)
            nc.vector.tensor_scalar(scr[:, :], g[:, :], 1.0, 0.0, AL.mult,
                                    AL.add, accum_out=out_t[:, ci:ci + 1])
        nc.vector.memset(out_t[:, 3:4], 0.0)
        nc.sync.dma_start(out_d[:, :], out_t[:, :])


_NC = None


def _get_nc():
    global _NC
    if _NC is None:
        nc = bacc.Bacc("TRN2", target_bir_lowering=False, debug=False,
                       num_devices=8)
        gt_d = nc.dram_tensor("gt", [128, FDH], U16,
                              kind="ExternalInput").ap()
        net_d = nc.dram_tensor("net", [128, 4 * FDO], F32,
                               kind="ExternalInput").ap()
        wn_d = nc.dram_tensor("wn", [128, 384], BF16,
                              kind="ExternalInput").ap()
        wp_d = nc.dram_tensor("wp", [128, 256], BF16,
                              kind="ExternalInput").ap()
        out_d = nc.dram_tensor("out", [128, 4], F32,
                               kind="ExternalOutput").ap()
        x_d = nc.dram_tensor("xs", [128, 3 * FDO], U16,
                             kind="ExternalOutput").ap()
        with TileContext(nc) as tc:
            _body(tc, gt_d, net_d, wn_d, wp_d, out_d, x_d)
        nc.compile()
        _NC = nc
    return _NC


def _in_maps(net_output, gt):
    bf = ml_dtypes.bfloat16
    b0 = (np.eye(128) + LB * (np.eye(128, k=1) + np.eye(128, k=-1))
          + LB ** 4 * (np.eye(128, k=2) + np.eye(128, k=-2)))
    wn = np.concatenate([b0, LB * b0, LB ** 4 * b0], axis=1).astype(bf)
    p0 = np.eye(128) + LB * (np.eye(128, k=1) + np.eye(128, k=-1))
    wp = np.concatenate([p0, LB * p0], axis=1).astype(bf)

    gtu = np.asarray(gt)[:, 0].astype(np.uint16)
    gtz = np.pad(gtu, ((0, 0), (0, 0), (0, 0), (H, H)), constant_values=255)
    maps = []
    for core in range(8):
        b, zs = core // 4, core % 4
        z0 = zs * ZO
        sl = gtz[b, :, :, z0:z0 + ZT]                       # [128, 128, 28]
        gts = np.pad(sl, ((0, 0), (2, 2), (0, 0)), constant_values=255)
        nets = np.ascontiguousarray(
            np.transpose(net_output[b, :, :, :, z0:z0 + ZO], (1, 0, 2, 3)))
        maps.append({
            "gt": gts.reshape(128, FDH),
            "net": nets.reshape(128, 4 * FDO).astype(np.float32),
            "wn": wn, "wp": wp,
        })
    return maps


def _pos_window_ok(gtu):
    """True iff no foreground voxel (any class 1..3) has its entire 3^3
    neighborhood foreground-of-the-same-class (pos2 <= 3 everywhere,
    out-of-volume treated as foreground)."""
    for c in range(1, C):
        m = gtu == c
        p = np.pad(m, ((0, 0), (1, 1), (1, 1), (1, 1)), constant_values=True)
        ex = p[:, :-2] & p[:, 1:-1] & p[:, 2:]
        ey = ex[:, :, :-2] & ex[:, :, 1:-1] & ex[:, :, 2:]
        ez = ey[:, :, :, :-2] & ey[:, :, :, 1:-1] & ey[:, :, :, 2:]
        if (m & ez).any():
            return False
    return True


def _fallback(net_output, gt):
    """Exact host computation (safety net if windowed-EDT verification
    fails)."""
    from scipy import ndimage
    net = np.asarray(net_output, np.float64)
    g = np.asarray(gt)[:, 0]
    e = np.exp(net - net.max(axis=1, keepdims=True))
    probs = e / e.sum(axis=1, keepdims=True)
    tot = 0.0
    for b in range(B):
        for c in range(1, C):
            m = g[b] == c
            if not m.any():
                continue
            pos = ndimage.distance_transform_edt(m)
            neg = ndimage.distance_transform_edt(~m)
            er = ndimage.binary_erosion(
                m, structure=ndimage.generate_binary_structure(3, 1),
                border_value=1)
            phi = np.where(m & ~er, 0.0, neg - pos)
            tot += float((probs[b, c] * phi).sum())
    return np.float32(tot / NVOX)


def kernel(net_output, gt, _spmd_result=[None]):
    nc = _get_nc()
    res = bass_utils.run_bass_kernel_spmd(nc, _in_maps(net_output, gt),
                                          core_ids=list(range(8)))
    _spmd_result[0] = res
    total, ok = 0.0, True
    for r in res.results:
        o = np.asarray(r["out"], np.float64)
        total += o[:, 0:3].sum()
        xs = np.asarray(r["xs"])
        ok &= bool(xs.min() >= 8)       # x = 16 - m; need m <= 8 everywhere
    ok = ok and _pos_window_ok(np.asarray(gt)[:, 0])
    if not ok:
        return _fallback(net_output, gt)
    return np.float32(total / NVOX)


# revision 19
# speedup vs baseline: 1.4663x; 1.1279x over previous
"""Boundary-distance loss (BDLoss) on 8 Trainium2 NeuronCores.

Math (matches the reference):
  probs = softmax(net_output, axis=1)
  onehot_c = (gt == c)
  posdis = EDT(onehot_c), negdis = EDT(~onehot_c)
  phi = where(inner_boundary, 0, negdis - posdis), zeroed if class absent
  out  = mean(probs[:, 1:] * phi[:, 1:])

Algorithm (exponential-weight separable convolution):
  * NEG field: E = conv3d(fpos, beta^-d^2) over the 5^3 box with beta = 2^8.
    Since all weights are powers of two and counts per distance-shell are
    < 32, the EXPONENT FIELD of E encodes m = min d^2 exactly:
    x := (bits(E) + 512) >> 10 == 16 - m (x == 0 for "nothing in box").
    The conv is separable: the x-taps ride in banded 128x128 matmul weights,
    the y-taps are 5 shifted matmul passes accumulated in PSUM (all on PE),
    and the z-taps are 4 pair-adds + 2 scales on DVE.  sqrt(m) is a single
    ACT pass: sqrt(-1*x + 16).
  * POS field: posE = conv3d(fneg, beta^-d^2) over the 3^3 box (fneg = "bg
    for class c", with OOV/pads forced to 0 so out-of-volume never counts
    as background).  Then the boundary-zeroed positive distance is
    posd = sqrt2*[posE < 2^-9] + (sqrt3-sqrt2)*[posE < 2^-17]
    (thresholds = "no bg at d^2<=1 / <=2"), auto-zero on bg voxels because
    the center tap makes posE >= 1 there.
  * phi = sqrt(m) - posd; result = sum(probs_c * phi) via a DVE
    tensor_scalar with accum_out per class; host sums across cores.
  * Verification: x-fields are shipped; min(x) >= 8 iff the windowed box
    EDT is exact (neg^2 <= 8 everywhere); pos exactness (pos^2 <= 3) is
    checked with numpy erosion.  On failure -> exact scipy fallback.

Sharding: core = (b, z-slab): b = core//4, z0 = 24*(core%4).  gt is sent as
uint16 with a 2-plane z halo and 2-column y pad of value 255.
"""

import numpy as np
import ml_dtypes

import concourse.bacc as bacc
import concourse.mybir as mybir
from concourse.tile import TileContext
from concourse import bass_utils

F32 = mybir.dt.float32
BF16 = mybir.dt.bfloat16
U16 = mybir.dt.uint16
AL = mybir.AluOpType
AF = mybir.ActivationFunctionType

B, C, X, Y, Z = 2, 4, 128, 128, 96
ZO = 24            # output z-planes per core
H = 2              # z halo
ZT = ZO + 2 * H    # 28 z-planes on chip
YP = Y + 4         # y padded to 132
FDH = YP * ZT      # 3696 cols of the padded mask tile
FDO = Y * ZO       # 3072 cols of a dense output tile
NVOX = B * (C - 1) * X * Y * Z
SQ2 = float(np.sqrt(2.0))
SQ3 = float(np.sqrt(3.0))
LB = 2.0 ** -8     # conv base beta^-1
T1 = 2.0 ** -9     # pos threshold: no bg at d^2 <= 1
T2 = 2.0 ** -17    # pos threshold: no bg at d^2 <= 2
CW = 3584          # conv output cols: y in [2,130) x z in [0,28)
C_LO = 2 * ZT      # first output col
CHUNK = 1024       # psum chunk (2 banks)
SUB = 512          # matmul sub-block (1 bank)

NEG_PASSES = [(2, -2), (2, 2), (1, -1), (1, 1), (0, 0)]   # (wn band idx, dy)
POS_PASSES = [(1, -1), (1, 1), (0, 0)]                    # (wp band idx, dy)


def _body(tc, gt_d, net_d, wn_d, wp_d, out_d, x_d):
    nc = tc.nc
    with tc.tile_pool(name="main", bufs=1) as pool, \
         tc.tile_pool(name="rot", bufs=2) as rot, \
         tc.tile_pool(name="nps", bufs=2, space="PSUM") as nps, \
         tc.tile_pool(name="pps", bufs=2, space="PSUM") as pps:

        # ---- input loads (weights first: they feed the PE warm-up) ---
        wn_t = pool.tile([128, 384], BF16, tag="wn")
        nc.sync.dma_start(wn_t[:, :], wn_d)
        wp_t = pool.tile([128, 256], BF16, tag="wp")
        nc.sync.dma_start(wp_t[:, :], wp_d)
        gt16 = pool.tile([128, FDH], U16, tag="gt")
        for gg in range(2):
            sl = slice(gg * FDH // 2, (gg + 1) * FDH // 2)
            nc.sync.dma_start(gt16[:, sl], gt_d[:, sl])
        b16 = pool.tile([128, 1], F32, tag="b16")
        nc.vector.memset(b16[:, :], 16.0)

        # ---- softmax exps: 2 now, 6 after class-1 copy emission ------
        e_t = pool.tile([128, 4 * FDO], BF16, tag="et")
        HF = FDO // 2

        def emit_exps(rng):
            for cc in rng:
                st = rot.tile([128, HF], F32, tag="stage", bufs=2)
                nc.sync.dma_start(st[:, :], net_d[:, cc * HF:(cc + 1) * HF])
                nc.scalar.activation(e_t[:, cc * HF:(cc + 1) * HF], st[:, :],
                                     AF.Exp)

        emit_exps(range(2))

        # ---- PE warm-up: keep the tensor engine busy from t~0.6us so
        # the p-state model ramps to full speed before the real convs
        # (a stalled wait resets the ramp; these chain into chunk 0's
        # psum, which the first real pass resets via start=True).
        warm = nps.tile([128, CHUNK], F32, tag="exyps", bufs=2)
        for _ in range(10):
            nc.tensor.matmul(warm[:, 0:384], wn_t[:, 0:128], wn_t[:, :],
                             start=True, stop=True)

        gtv = gt16[:, :].rearrange("p (y z) -> p y z", z=ZT)

        def build_masks(c, nch=1):
            fpos = rot.tile([128, FDH], BF16, tag="fpos", bufs=3)
            fneg = rot.tile([128, FDH], BF16, tag="fneg", bufs=3)
            for g in range(nch):
                sl = slice(g * FDH // nch, (g + 1) * FDH // nch)
                nc.vector.tensor_scalar(fpos[:, sl], gt16[:, sl], c, None,
                                        AL.is_equal)
            for g in range(nch):
                sl = slice(g * FDH // nch, (g + 1) * FDH // nch)
                nc.vector.tensor_scalar(fneg[:, sl], gt16[:, sl], c, None,
                                        AL.not_equal)
            # y pads -> 0 (OOV is never background)
            nc.gpsimd.memset(fneg[:, 0:2 * ZT], 0.0)
            nc.gpsimd.memset(fneg[:, 130 * ZT:132 * ZT], 0.0)
            return fpos, fneg

        def halo_fix(fpos, fneg):
            # z halo -> V - fpos (0 on OOV pad, unchanged on real data)
            fnv = fneg[:, :].rearrange("p (y z) -> p y z", z=ZT)
            fpv = fpos[:, :].rearrange("p (y z) -> p y z", z=ZT)
            nc.vector.tensor_tensor(fnv[:, :, 0:2], vhv[:, :, 0:2],
                                    fpv[:, :, 0:2], AL.subtract)
            nc.vector.tensor_tensor(fnv[:, :, ZT - 2:ZT], vhv[:, :, 2:4],
                                    fpv[:, :, ZT - 2:ZT], AL.subtract)

        den = pool.tile([128, FDO], BF16, tag="den")
        inv_t = pool.tile([128, FDO], BF16, tag="inv")
        out_t = pool.tile([128, 4], F32, tag="out")
        scr = e_t[:, 0:FDO]        # e0 slot is dead once den exists
        xzs, gs = {}, {}
        masks = [build_masks(1, nch=2)]
        # valid-mask z-halo planes (for the fneg halo fix), after masks so
        # the DVE queue serves class-1's masks first
        vh = pool.tile([128, 4 * YP], BF16, tag="vh")
        vhv = vh[:, :].rearrange("p (y z) -> p y z", z=4)
        nc.vector.tensor_scalar(vhv[:, :, 0:2], gtv[:, :, 0:2], 3, None,
                                AL.is_le)
        nc.vector.tensor_scalar(vhv[:, :, 2:4], gtv[:, :, ZT - 2:ZT], 3,
                                None, AL.is_le)
        halo_fix(*masks[0])

        def conv_xy(mask, passes, wmat, tag, ppool):
            """x-band (x) y-shift (xy) conv passes -> SBUF bf16 [128, CW]."""
            sb = rot.tile([128, CW], BF16, tag=tag, bufs=2)
            off = 0
            while off < CW:
                w = min(CHUNK, CW - off)
                ps = ppool.tile([128, CHUNK], F32, tag=tag + "ps", bufs=2)
                for pi, (wi, dy) in enumerate(passes):
                    first, last = pi == 0, pi == len(passes) - 1
                    for mm in range(0, w, SUB):
                        mw = min(SUB, w - mm)
                        a = C_LO + dy * ZT + off + mm
                        nc.tensor.matmul(ps[:, mm:mm + mw],
                                         wmat[:, 128 * wi:128 * (wi + 1)],
                                         mask[:, a:a + mw],
                                         start=first, stop=last)
                nc.scalar.activation(sb[:, off:off + w], ps[:, 0:w], AF.Copy)
                off += w
            return sb

        YH = Y // 2

        def zconv5(sb):
            """5-tap z-conv on DVE (y-halves) -> flat [128, FDO] bf16."""
            v = sb[:, :].rearrange("p (y z) -> p y z", z=ZT)
            a = rot.tile([128, FDO], BF16, tag="A", bufs=2)
            b = rot.tile([128, FDO], BF16, tag="Bt", bufs=2)
            av = a[:, :].rearrange("p (y z) -> p y z", z=ZO)
            bv = b[:, :].rearrange("p (y z) -> p y z", z=ZO)
            for y0 in (0, YH):
                ys = slice(y0, y0 + YH)
                nc.vector.tensor_tensor(av[:, ys, :], v[:, ys, 1:1 + ZO],
                                        v[:, ys, 3:3 + ZO], AL.add)
                nc.vector.tensor_tensor(bv[:, ys, :], v[:, ys, 0:ZO],
                                        v[:, ys, 4:4 + ZO], AL.add)
                nc.vector.tensor_scalar(av[:, ys, :], av[:, ys, :], LB,
                                        None, AL.mult)
                nc.vector.tensor_scalar(bv[:, ys, :], bv[:, ys, :], LB ** 4,
                                        None, AL.mult)
                nc.vector.tensor_tensor(av[:, ys, :], v[:, ys, 2:2 + ZO],
                                        av[:, ys, :], AL.add)
                nc.vector.tensor_tensor(av[:, ys, :], av[:, ys, :],
                                        bv[:, ys, :], AL.add)
            return a, b

        def zconv3(sb):
            """3-tap z-conv on DVE (y-halves) -> flat [128, FDO] bf16."""
            v = sb[:, :].rearrange("p (y z) -> p y z", z=ZT)
            a = rot.tile([128, FDO], BF16, tag="Bt", bufs=2)
            av = a[:, :].rearrange("p (y z) -> p y z", z=ZO)
            for y0 in (0, YH):
                ys = slice(y0, y0 + YH)
                nc.vector.tensor_tensor(av[:, ys, :], v[:, ys, 1:1 + ZO],
                                        v[:, ys, 3:3 + ZO], AL.add)
                nc.vector.tensor_scalar(av[:, ys, :], av[:, ys, :], LB,
                                        None, AL.mult)
                nc.vector.tensor_tensor(av[:, ys, :], v[:, ys, 2:2 + ZO],
                                        av[:, ys, :], AL.add)
            return a

        for ci in range(3):
            fpos, fneg = masks[ci]
            exy = conv_xy(fpos, NEG_PASSES, wn_t, "exy", nps)
            cxy = conv_xy(fneg, POS_PASSES, wp_t, "cxy", pps)
            if ci < 2:
                masks.append(build_masks(ci + 2))
                halo_fix(*masks[ci + 1])
            if ci == 0:
                emit_exps(range(2, 8))
                # den + 1/den + probability folds (Pool + ACT, off-path)
                nc.gpsimd.tensor_tensor(den[:, :], e_t[:, 0:FDO],
                                        e_t[:, FDO:2 * FDO], AL.add)
                nc.gpsimd.tensor_tensor(den[:, :], den[:, :],
                                        e_t[:, 2 * FDO:3 * FDO], AL.add)
                nc.gpsimd.tensor_tensor(den[:, :], den[:, :],
                                        e_t[:, 3 * FDO:4 * FDO], AL.add)
                for hh in range(2):
                    sl = slice(hh * HF, (hh + 1) * HF)
                    lh = rot.tile([128, HF], F32, tag="stage", bufs=2)
                    nc.scalar.activation(lh[:, :], den[:, sl], AF.Ln)
                    nc.scalar.activation(inv_t[:, sl], lh[:, :], AF.Exp,
                                         scale=-1.0)
                for c in (1, 2, 3):
                    esl = e_t[:, c * FDO:(c + 1) * FDO]
                    nc.gpsimd.tensor_tensor(esl, esl, inv_t[:, :], AL.mult)
            # neg z + exponent decode -> x = 16 - m (u16)
            ez, bt = zconv5(exy)
            xz = rot.tile([128, FDO], U16, tag="xz", bufs=3)
            nc.vector.tensor_scalar(bt[:, :].bitcast(U16),
                                    ez[:, :].bitcast(U16), 512, None, AL.add)
            nc.vector.tensor_scalar(xz[:, :], bt[:, :].bitcast(U16), 10,
                                    None, AL.logical_shift_right)
            nc.sync.dma_start(x_d[:, ci * FDO:(ci + 1) * FDO], xz[:, :])
            xzs[ci] = xz
            # pos z + thresholds -> g = posd'
            pe = zconv3(cxy)
            g1 = rot.tile([128, FDO], BF16, tag="g", bufs=3)
            g2 = rot.tile([128, FDO], BF16, tag="Bt", bufs=2)
            nc.vector.tensor_scalar(g1[:, :], pe[:, :], T1, SQ2, AL.is_lt,
                                    AL.mult)
            nc.vector.tensor_scalar(g2[:, :], pe[:, :], T2, SQ3 - SQ2,
                                    AL.is_lt, AL.mult)
            nc.vector.tensor_tensor(g1[:, :], g1[:, :], g2[:, :], AL.add)
            gs[ci] = g1

        # ---- sqrts batched (one act-table switch), then tails --------
        for ci in range(3):
            sn = rot.tile([128, FDO], BF16, tag="sn", bufs=2)
            nc.scalar.activation(sn[:, :], xzs[ci][:, :], AF.Sqrt,
                                 bias=b16[:, 0:1], scale=-1.0)
            g = gs[ci]
            nc.vector.tensor_tensor(g[:, :], sn[:, :], g[:, :],
                                    AL.subtract)
            nc.vector.tensor_tensor(g[:, :], g[:, :],
                                    e_t[:, (ci + 1) * FDO:(ci + 2) * FDO],
                                    AL.mult)
            nc.vector.tensor_scalar(scr[:, :], g[:, :], 1.0, 0.0, AL.mult,
                                    AL.add, accum_out=out_t[:, ci:ci + 1])
        nc.vector.memset(out_t[:, 3:4], 0.0)
        nc.sync.dma_start(out_d[:, :], out_t[:, :])


_NC = None


def _get_nc():
    global _NC
    if _NC is None:
        nc = bacc.Bacc("TRN2", target_bir_lowering=False, debug=False,
                       num_devices=8)
        gt_d = nc.dram_tensor("gt", [128, FDH], U16,
                              kind="ExternalInput").ap()
        net_d = nc.dram_tensor("net", [128, 4 * FDO], F32,
                               kind="ExternalInput").ap()
        wn_d = nc.dram_tensor("wn", [128, 384], BF16,
                              kind="ExternalInput").ap()
        wp_d = nc.dram_tensor("wp", [128, 256], BF16,
                              kind="ExternalInput").ap()
        out_d = nc.dram_tensor("out", [128, 4], F32,
                               kind="ExternalOutput").ap()
        x_d = nc.dram_tensor("xs", [128, 3 * FDO], U16,
                             kind="ExternalOutput").ap()
        with TileContext(nc) as tc:
            _body(tc, gt_d, net_d, wn_d, wp_d, out_d, x_d)
        nc.compile()
        _NC = nc
    return _NC


def _in_maps(net_output, gt):
    bf = ml_dtypes.bfloat16
    b0 = (np.eye(128) + LB * (np.eye(128, k=1) + np.eye(128, k=-1))
          + LB ** 4 * (np.eye(128, k=2) + np.eye(128, k=-2)))
    wn = np.concatenate([b0, LB * b0, LB ** 4 * b0], axis=1).astype(bf)
    p0 = np.eye(128) + LB * (np.eye(128, k=1) + np.eye(128, k=-1))
    wp = np.concatenate([p0, LB * p0], axis=1).astype(bf)

    gtu = np.asarray(gt)[:, 0].astype(np.uint16)
    gtz = np.pad(gtu, ((0, 0), (0, 0), (0, 0), (H, H)), constant_values=255)
    maps = []
    for core in range(8):
        b, zs = core // 4, core % 4
        z0 = zs * ZO
        sl = gtz[b, :, :, z0:z0 + ZT]                       # [128, 128, 28]
        gts = np.pad(sl, ((0, 0), (2, 2), (0, 0)), constant_values=255)
        nets = np.ascontiguousarray(
            np.transpose(net_output[b, :, :, :, z0:z0 + ZO], (1, 0, 2, 3)))
        maps.append({
            "gt": gts.reshape(128, FDH),
            "net": nets.reshape(128, 4 * FDO).astype(np.float32),
            "wn": wn, "wp": wp,
        })
    return maps


def _pos_window_ok(gtu):
    """True iff no foreground voxel (any class 1..3) has its entire 3^3
    neighborhood foreground-of-the-same-class (pos2 <= 3 everywhere,
    out-of-volume treated as foreground)."""
    for c in range(1, C):
        m = gtu == c
        p = np.pad(m, ((0, 0), (1, 1), (1, 1), (1, 1)), constant_values=True)
        ex = p[:, :-2] & p[:, 1:-1] & p[:, 2:]
        ey = ex[:, :, :-2] & ex[:, :, 1:-1] & ex[:, :, 2:]
        ez = ey[:, :, :, :-2] & ey[:, :, :, 1:-1] & ey[:, :, :, 2:]
        if (m & ez).any():
            return False
    return True


def _fallback(net_output, gt):
    """Exact host computation (safety net if windowed-EDT verification
    fails)."""
    from scipy import ndimage
    net = np.asarray(net_output, np.float64)
    g = np.asarray(gt)[:, 0]
    e = np.exp(net - net.max(axis=1, keepdims=True))
    probs = e / e.sum(axis=1, keepdims=True)
    tot = 0.0
    for b in range(B):
        for c in range(1, C):
            m = g[b] == c
            if not m.any():
                continue
            pos = ndimage.distance_transform_edt(m)
            neg = ndimage.distance_transform_edt(~m)
            er = ndimage.binary_erosion(
                m, structure=ndimage.generate_binary_structure(3, 1),
                border_value=1)
            phi = np.where(m & ~er, 0.0, neg - pos)
            tot += float((probs[b, c] * phi).sum())
    return np.float32(tot / NVOX)


def kernel(net_output, gt, _spmd_result=[None]):
    nc = _get_nc()
    res = bass_utils.run_bass_kernel_spmd(nc, _in_maps(net_output, gt),
                                          core_ids=list(range(8)))
    _spmd_result[0] = res
    total, ok = 0.0, True
    for r in res.results:
        o = np.asarray(r["out"], np.float64)
        total += o[:, 0:3].sum()
        xs = np.asarray(r["xs"])
        ok &= bool(xs.min() >= 8)       # x = 16 - m; need m <= 8 everywhere
    ok = ok and _pos_window_ok(np.asarray(gt)[:, 0])
    if not ok:
        return _fallback(net_output, gt)
    return np.float32(total / NVOX)
